# revision 16
# baseline (speedup 1.0000x reference)
"""Trainium2 Bass kernel for a LoRA self-attention block (diffusers-style
CustomLoRAAttnProcessor).

  B=8, S=1024, D=1280, H=20 heads x HD=64, LoRA rank 4 (folded into the
  weights on the host: W_eff = W + 0.25 * B @ A, mathematically identical).

Distribution: pure data parallelism — one batch element per NeuronCore
(8 cores), no collectives.

Per-core kernel (all layouts chosen so the contraction dim sits on SBUF
partitions; host pre-transposes x and the effective weights):

  phase 1: qT = WqT.T-style matmuls -> qT/kT feature-major [D, S];
           v token-major [S, D] with a ones-column appended per head
           (so the AV matmul also produces the softmax denominator).
  phase 2: per head pair: scoresT[k,q] = kT_h.T-slices @ qT_h (K=64 row
           pairs pack the PE array), exp on ACT (scale=1/8 folded in,
           no max-subtraction needed: |scores| <= ~6), AV+sumexp via the
           [v_h | 1] stationary operand, normalize with a broadcast
           reciprocal.
  phase 3: outT = WoT-style matmul over ctxT + bias, DMA out.
"""

import sys

for _p in ("/opt/trn_rl_repo",):
    if _p not in sys.path:
        sys.path.insert(0, _p)

from contextlib import ExitStack

import numpy as np

import concourse.bass as bass  # noqa: F401  (import order: bass before tile)
import concourse.tile as tile
from concourse import bacc, mybir
from concourse.bass_utils import run_bass_kernel_spmd


def _patch_act_tables():
    """Force Exp and Ln onto the single combined table set so the
    insert_act_table_loads fixpoint emits one load instead of ping-ponging
    between exp_and_others and natural_log (2.7us per switch)."""
    from concourse import hw_specs

    orig = hw_specs.get_activation_tables
    combined = "natural_log_exp_and_others"

    def patched(module_arch):
        tables = orig(module_arch)
        if combined in tables:
            for name, fns in tables.items():
                if name != combined:
                    fns.discard(mybir.ActivationFunctionType.Exp)
                    fns.discard(mybir.ActivationFunctionType.Ln)
        return tables

    hw_specs.get_activation_tables = patched
    bacc.get_activation_tables = patched


_patch_act_tables()

B, S, D = 8, 1024, 1280
H, HD = 20, 64
SCALING = 0.25  # alpha / rank
ATTN_SCALE = 1.0 / 8.0  # 1/sqrt(HD)

DT = D // 128  # 10 feature tiles
KC = S // 128  # 8 key-position chunks
MG = 5  # weight column groups of 256 (2 output tiles each)
VW = HD + 1  # v columns per head incl. ones column

F32 = mybir.dt.float32
BF16 = mybir.dt.bfloat16
EXP = mybir.ActivationFunctionType.Exp
LN = mybir.ActivationFunctionType.Ln

N_CORES = 8


def _emit(nc, tc, xT, wqT, wkT, wvT, woT, bo, outT, taps=None):
    persist_cm = tc.tile_pool(name="persist", bufs=1)
    persist = persist_cm.__enter__()
    qT_sb = persist.tile([128, DT, S], F32)
    kT_sb = persist.tile([128, DT, S], F32)
    v_sb = persist.tile([128, KC, H * VW], BF16)
    ctxT_sb = persist.tile([128, DT, S], F32)
    bo_sb = persist.tile([128, DT, 1], F32)
    nc.sync.dma_start(out=bo_sb, in_=bo[:].rearrange("(t p) -> p t", p=128))
    # ones columns for the AV+sumexp trick
    nc.vector.memset(
        v_sb[:].rearrange("p a (h c) -> p a h c", c=VW)[:, :, :, HD : HD + 1], 1.0
    )

    # ---------------- phase 1: projections ----------------
    with ExitStack() as p1:
        xpool = p1.enter_context(tc.tile_pool(name="xpool", bufs=1))
        wpool = p1.enter_context(tc.tile_pool(name="wpool", bufs=2))
        pp = p1.enter_context(tc.tile_pool(name="pp", bufs=4, space="PSUM"))
        vp = p1.enter_context(tc.tile_pool(name="vp", bufs=4, space="PSUM"))

        xT_sb = xpool.tile([128, DT, S], F32)
        for t in range(DT):
            nc.sync.dma_start(
                out=xT_sb[:, t, :],
                in_=xT[t * 128 : (t + 1) * 128, :],
            )

        # q and k, feature-major output [D, S]
        for wdram, dst in ((wqT, qT_sb), (wkT, kT_sb)):
            for mg in range(MG):
                stripe = wpool.tile([128, DT, 256], F32, tag="w")
                nc.sync.dma_start(
                    out=stripe,
                    in_=wdram[:, mg * 256 : (mg + 1) * 256].rearrange(
                        "(t p) n -> p t n", p=128
                    ),
                )
                for ml in range(2):
                    m = mg * 2 + ml
                    for qc in range(2):
                        ps = pp.tile([128, 512], F32, tag="pp")
                        for kk in range(DT):
                            nc.tensor.matmul(
                                ps,
                                lhsT=stripe[:, kk, ml * 128 : (ml + 1) * 128],
                                rhs=xT_sb[:, kk, qc * 512 : (qc + 1) * 512],
                                start=(kk == 0),
                                stop=(kk == DT - 1),
                            )
                        nc.vector.tensor_copy(
                            out=dst[:, m, qc * 512 : (qc + 1) * 512], in_=ps
                        )

        # v, token-major output [S, D] (bf16, interleaved with ones columns)
        for ng in range(MG):
            stripe = wpool.tile([128, DT, 256], F32, tag="w")
            nc.sync.dma_start(
                out=stripe,
                in_=wvT[:, ng * 256 : (ng + 1) * 256].rearrange(
                    "(t p) n -> p t n", p=128
                ),
            )
            for st in range(KC):
                ps = vp.tile([128, 256], F32, tag="vp")
                for kk in range(DT):
                    nc.tensor.matmul(
                        ps,
                        lhsT=xT_sb[:, kk, st * 128 : (st + 1) * 128],
                        rhs=stripe[:, kk, :],
                        start=(kk == 0),
                        stop=(kk == DT - 1),
                    )
                nc.vector.tensor_copy(
                    out=v_sb[:, st, ng * 4 * VW : (ng + 1) * 4 * VW].rearrange(
                        "p (h c) -> p h c", c=VW
                    )[:, :, 0:HD],
                    in_=ps[:].rearrange("p (h c) -> p h c", c=HD),
                )

    if taps is not None:
        nc.sync.dma_start(out=taps["qT_dump"][:], in_=qT_sb)
        nc.sync.dma_start(out=taps["kT_dump"][:], in_=kT_sb)
        nc.sync.dma_start(out=taps["v_dump"][:], in_=v_sb)

    # ---------------- phase 2: attention ----------------
    with ExitStack() as p2:
        epool = p2.enter_context(tc.tile_pool(name="epool", bufs=2))
        spool = p2.enter_context(tc.tile_pool(name="sc_ps", bufs=2, space="PSUM"))
        cpool = p2.enter_context(tc.tile_pool(name="ctx_ps", bufs=2, space="PSUM"))
        small = p2.enter_context(tc.tile_pool(name="small", bufs=2))

        for t in range(DT):  # head pairs (2t, 2t+1) share feature tile t
            exps = [
                epool.tile([128, KC, S], BF16, tag="exp", name=f"exp{t}_{i}")
                for i in range(2)
            ]
            for kc in range(KC):
                for half in range(2):
                    p0 = half * 64
                    ps = spool.tile([128, S], F32, tag="sc")
                    for qc in range(2):
                        nc.tensor.matmul(
                            ps[:, qc * 512 : (qc + 1) * 512],
                            lhsT=kT_sb[p0 : p0 + 64, t, kc * 128 : (kc + 1) * 128],
                            rhs=qT_sb[p0 : p0 + 64, t, qc * 512 : (qc + 1) * 512],
                            start=True,
                            stop=True,
                        )
                    nc.scalar.activation(
                        out=exps[half][:, kc, :], in_=ps, func=EXP, scale=ATTN_SCALE
                    )
            if taps is not None and t == 0:
                for half in range(2):
                    nc.sync.dma_start(
                        out=taps["exp_dump"][half], in_=exps[half]
                    )
            for half in range(2):
                h = 2 * t + half
                ctx_ps = cpool.tile([HD + 1, S], F32, tag="ctx")
                for qc in range(2):
                    for kc in range(KC):
                        nc.tensor.matmul(
                            ctx_ps[:, qc * 512 : (qc + 1) * 512],
                            lhsT=v_sb[:, kc, h * VW : (h + 1) * VW],
                            rhs=exps[half][:, kc, qc * 512 : (qc + 1) * 512],
                            start=(kc == 0),
                            stop=(kc == KC - 1),
                        )
                if taps is not None and t == 0:
                    tap_sb = small.tile([HD + 1, S], F32, tag="tap", name="tap_sb")
                    nc.vector.tensor_copy(out=tap_sb, in_=ctx_ps)
                    nc.sync.dma_start(out=taps["ctxp_dump"][half], in_=tap_sb)
                # 1/sumexp = exp(-ln(sumexp)) on ACT: Ln and Exp share the
                # natural_log_exp_and_others table set, and the custom-DVE
                # reciprocal path does not work on this runtime.
                lnrow = small.tile([1, S], F32, tag="lnrow", name=f"ln{t}_{half}")
                nc.scalar.activation(
                    out=lnrow, in_=ctx_ps[HD : HD + 1, :], func=LN
                )
                recip = small.tile([1, S], F32, tag="recip")
                nc.scalar.activation(out=recip, in_=lnrow, func=EXP, scale=-1.0)
                bcast = small.tile([HD, S], F32, tag="bcast")
                nc.gpsimd.partition_broadcast(bcast, recip)
                if taps is not None and t == 0:
                    nc.sync.dma_start(out=taps["recip_dump"][half], in_=recip)
                    nc.sync.dma_start(out=taps["bcast_dump"][half], in_=bcast)
                nc.vector.tensor_mul(
                    ctxT_sb[half * 64 : half * 64 + 64, t, :],
                    ctx_ps[0:HD, :],
                    bcast,
                )

    if taps is not None:
        nc.sync.dma_start(out=taps["ctxT_dump"][:], in_=ctxT_sb)

    # ---------------- phase 3: output projection ----------------
    with ExitStack() as p3:
        wpool3 = p3.enter_context(tc.tile_pool(name="wpool3", bufs=3))
        opp = p3.enter_context(tc.tile_pool(name="opp", bufs=4, space="PSUM"))
        ostage = p3.enter_context(tc.tile_pool(name="ostage", bufs=4))
        for mg in range(MG):
            stripe = wpool3.tile([128, DT, 256], F32, tag="w3")
            nc.sync.dma_start(
                out=stripe,
                in_=woT[:, mg * 256 : (mg + 1) * 256].rearrange(
                    "(t p) n -> p t n", p=128
                ),
            )
            for ml in range(2):
                m = mg * 2 + ml
                for qc in range(2):
                    ps = opp.tile([128, 512], F32, tag="opp")
                    for kk in range(DT):
                        nc.tensor.matmul(
                            ps,
                            lhsT=stripe[:, kk, ml * 128 : (ml + 1) * 128],
                            rhs=ctxT_sb[:, kk, qc * 512 : (qc + 1) * 512],
                            start=(kk == 0),
                            stop=(kk == DT - 1),
                        )
                    o_sb = ostage.tile([128, 512], F32, tag="ostage")
                    nc.vector.tensor_scalar_add(o_sb, ps, bo_sb[:, m, :])
                    nc.sync.dma_start(
                        out=outT[m * 128 : (m + 1) * 128, qc * 512 : (qc + 1) * 512],
                        in_=o_sb,
                    )
    persist_cm.__exit__(None, None, None)


def build_nc(debug_taps=False):
    nc = bacc.Bacc(None, target_bir_lowering=False)
    xT = nc.dram_tensor("xT", [D, S], F32, kind="ExternalInput")
    wqT = nc.dram_tensor("wqT", [D, D], F32, kind="ExternalInput")
    wkT = nc.dram_tensor("wkT", [D, D], F32, kind="ExternalInput")
    wvT = nc.dram_tensor("wvT", [D, D], F32, kind="ExternalInput")
    woT = nc.dram_tensor("woT", [D, D], F32, kind="ExternalInput")
    bo = nc.dram_tensor("bo", [D], F32, kind="ExternalInput")
    outT = nc.dram_tensor("outT", [D, S], F32, kind="ExternalOutput")
    taps = None
    if debug_taps:
        taps = {
            "qT_dump": nc.dram_tensor("qT_dump", [128, DT, S], F32, kind="ExternalOutput"),
            "kT_dump": nc.dram_tensor("kT_dump", [128, DT, S], F32, kind="ExternalOutput"),
            "v_dump": nc.dram_tensor("v_dump", [128, KC, H * VW], BF16, kind="ExternalOutput"),
            "exp_dump": nc.dram_tensor("exp_dump", [2, 128, KC, S], BF16, kind="ExternalOutput"),
            "ctxp_dump": nc.dram_tensor("ctxp_dump", [2, HD + 1, S], F32, kind="ExternalOutput"),
            "ctxT_dump": nc.dram_tensor("ctxT_dump", [128, DT, S], F32, kind="ExternalOutput"),
            "recip_dump": nc.dram_tensor("recip_dump", [2, 1, S], F32, kind="ExternalOutput"),
            "bcast_dump": nc.dram_tensor("bcast_dump", [2, HD, S], F32, kind="ExternalOutput"),
        }
    with tile.TileContext(nc) as tc:
        _emit(nc, tc, xT, wqT, wkT, wvT, woT, bo, outT, taps=taps)
    nc.compile()
    return nc


_NC = None


def _get_nc():
    global _NC
    if _NC is None:
        _NC = build_nc()
    return _NC


def make_in_maps(hidden_states, Wq, Wk, Wv, Wo, bo, Aq, Bq, Ak, Bk, Av, Bv, Ao, Bo):
    x = np.asarray(hidden_states, dtype=np.float32)

    def eff_T(W, A, Bup):
        W64 = np.asarray(W, dtype=np.float64)
        lora = np.asarray(Bup, dtype=np.float64) @ np.asarray(A, dtype=np.float64)
        return np.ascontiguousarray((W64 + SCALING * lora).T.astype(np.float32))

    base = {
        "wqT": eff_T(Wq, Aq, Bq),
        "wkT": eff_T(Wk, Ak, Bk),
        "wvT": eff_T(Wv, Av, Bv),
        "woT": eff_T(Wo, Ao, Bo),
        "bo": np.ascontiguousarray(np.asarray(bo, dtype=np.float32)),
    }
    return [
        dict(base, xT=np.ascontiguousarray(x[b].T)) for b in range(x.shape[0])
    ]


def kernel(**inputs):
    in_maps = make_in_maps(**inputs)
    nc = _get_nc()
    res = run_bass_kernel_spmd(nc, in_maps, core_ids=list(range(N_CORES)))
    out = np.stack([res.results[b]["outT"].T for b in range(N_CORES)])
    return np.ascontiguousarray(out, dtype=np.float32)


# revision 17
# speedup vs baseline: 2.5601x; 2.5601x over previous
"""Trainium2 Bass kernel for a LoRA self-attention block (diffusers-style
CustomLoRAAttnProcessor).

  B=8, S=1024, D=1280, H=20 heads x HD=64, LoRA rank 4 (folded into the
  weights on the host: W_eff = W + 0.25 * B @ A, mathematically identical).

Distribution: pure data parallelism — one batch element per NeuronCore
(8 cores), no collectives.

Per-core layout choices (contraction dim always on SBUF partitions; host
pre-transposes x and the effective weights; all matmul operands bf16 —
fp32 matmuls cost 4x on TRN2):

  phase V : v token-major [S, D] with a ones-column per head (the AV
            matmul then also emits the softmax denominator for free).
  phase QK: qT/kT feature-major [D, S]; interleaved per weight column
            group with scoresT[k,q] (K=64 head pairs row-pack the PE via
            base_partition 0/64), exp on ACT (scale=1/8 folded, no
            max-subtraction: |scores| <= ~6), AV + normalize (reciprocal
            via DMA-reshape so 64 DVE lanes share the work).
  phase O : outT = WoT-style matmul over ctxT + bias, DMA out.
"""

import sys

for _p in ("/opt/trn_rl_repo",):
    if _p not in sys.path:
        sys.path.insert(0, _p)

from contextlib import ExitStack

import ml_dtypes
import numpy as np

import concourse.bass as bass  # noqa: F401  (import order: bass before tile)
import concourse.tile as tile
from concourse import bacc, mybir
from concourse.bass_utils import run_bass_kernel_spmd

B, S, D = 8, 1024, 1280
H, HD = 20, 64
SCALING = 0.25  # alpha / rank
ATTN_SCALE = 1.0 / 8.0  # 1/sqrt(HD)

DT = D // 128  # 10 feature tiles
KC = S // 128  # 8 key-position chunks
MG = 5  # weight column groups of 256 (2 output tiles each)
VW = HD + 1  # v columns per head incl. ones column

F32 = mybir.dt.float32
BF16 = mybir.dt.bfloat16
EXP = mybir.ActivationFunctionType.Exp

N_CORES = 8


def _qk_mgroup(nc, xT_sb, wpool, pp, wdram, dst, mg):
    """One 256-wide column group of a feature-major projection:
    dst[:, m, :] = (W.T @ x.T) for m in the group. Stationary weight tile
    is reused across both q-column halves (2 matmuls per LDWEIGHTS)."""
    stripe = wpool.tile([128, DT, 256], BF16, tag="w", name=f"w{mg}")
    nc.sync.dma_start(
        out=stripe,
        in_=wdram[:, mg * 256 : (mg + 1) * 256].rearrange("(t p) n -> p t n", p=128),
    )
    for ml in range(2):
        m = mg * 2 + ml
        ps0 = pp.tile([128, 512], F32, tag="pp", name=f"ps0_{m}")
        ps1 = pp.tile([128, 512], F32, tag="pp", name=f"ps1_{m}")
        for kk in range(DT):
            lhsT = stripe[:, kk, ml * 128 : (ml + 1) * 128]
            nc.tensor.matmul(
                ps0, lhsT=lhsT, rhs=xT_sb[:, kk, 0:512],
                start=(kk == 0), stop=(kk == DT - 1),
            )
            nc.tensor.matmul(
                ps1, lhsT=lhsT, rhs=xT_sb[:, kk, 512:1024],
                start=(kk == 0), stop=(kk == DT - 1),
            )
        nc.vector.tensor_copy(out=dst[:, m, 0:512], in_=ps0)
        nc.vector.tensor_copy(out=dst[:, m, 512:1024], in_=ps1)


def _emit(nc, tc, xT, wqT, wkT, wvT, woT, bo, outT):
    persist_cm = tc.tile_pool(name="persist", bufs=1)
    persist = persist_cm.__enter__()
    qT_sb = persist.tile([128, DT, S], BF16)
    kT_sb = persist.tile([128, DT, S], BF16)
    v_sb = persist.tile([128, KC, H * VW], BF16)
    ctxT_sb = persist.tile([128, DT, S], BF16)
    bo_sb = persist.tile([128, DT, 1], F32)
    nc.sync.dma_start(out=bo_sb, in_=bo[:].rearrange("(t p) -> p t", p=128))
    nc.vector.memset(
        v_sb[:].rearrange("p a (h c) -> p a h c", c=VW)[:, :, :, HD : HD + 1], 1.0
    )

    xpool_cm = tc.tile_pool(name="xpool", bufs=1)
    xpool = xpool_cm.__enter__()
    xT_sb = xpool.tile([128, DT, S], BF16)
    for t in range(DT):
        nc.sync.dma_start(out=xT_sb[:, t, :], in_=xT[t * 128 : (t + 1) * 128, :])

    # ---------------- phase V: v projection (token-major) ----------------
    with ExitStack() as pv:
        vwpool = pv.enter_context(tc.tile_pool(name="vwpool", bufs=1))
        vp = pv.enter_context(tc.tile_pool(name="vp", bufs=4, space="PSUM"))
        vw = vwpool.tile([128, DT, D], BF16)
        for kk in range(DT):
            nc.sync.dma_start(
                out=vw[:, kk, :], in_=wvT[kk * 128 : (kk + 1) * 128, :]
            )
        NCH = ((0, 512), (512, 512), (1024, 256))
        for st in range(KC):
            pss = [
                vp.tile([128, nw], F32, tag="vp", name=f"vps{st}_{ni}")
                for ni, (n0, nw) in enumerate(NCH)
            ]
            for kk in range(DT):
                lhsT = xT_sb[:, kk, st * 128 : (st + 1) * 128]
                for ni, (n0, nw) in enumerate(NCH):
                    nc.tensor.matmul(
                        pss[ni], lhsT=lhsT, rhs=vw[:, kk, n0 : n0 + nw],
                        start=(kk == 0), stop=(kk == DT - 1),
                    )
            for ni, (n0, nw) in enumerate(NCH):
                nh = nw // HD
                nc.vector.tensor_copy(
                    out=v_sb[
                        :, st, (n0 // HD) * VW : (n0 // HD + nh) * VW
                    ].rearrange("p (h c) -> p h c", c=VW)[:, :, 0:HD],
                    in_=pss[ni][:].rearrange("p (h c) -> p h c", c=HD),
                )

    # -------- phase QK + attention, interleaved per weight column group ----
    with ExitStack() as p2:
        wpool = p2.enter_context(tc.tile_pool(name="wpool", bufs=3))
        epool = p2.enter_context(tc.tile_pool(name="epool", bufs=3))
        small = p2.enter_context(tc.tile_pool(name="small", bufs=2))
        pp = p2.enter_context(tc.tile_pool(name="pp", bufs=2, space="PSUM"))
        sc = p2.enter_context(tc.tile_pool(name="sc", bufs=4, space="PSUM"))
        cx = p2.enter_context(tc.tile_pool(name="cx", bufs=2, space="PSUM"))

        for mg in range(MG):
            _qk_mgroup(nc, xT_sb, wpool, pp, wqT, qT_sb, mg)
            _qk_mgroup(nc, xT_sb, wpool, pp, wkT, kT_sb, mg)

            for t in (2 * mg, 2 * mg + 1):
                exps = [
                    epool.tile([128, KC, S], BF16, tag="exp", name=f"exp{t}_{i}")
                    for i in range(2)
                ]
                # scoresT + exp; head pair (2t, 2t+1) row-packs the PE
                for kc in range(KC):
                    for half in range(2):
                        p0 = half * 64
                        for qc in range(2):
                            ps = sc.tile([128, 512], F32, tag="sc", name="scps")
                            nc.tensor.matmul(
                                ps,
                                lhsT=kT_sb[p0 : p0 + 64, t, kc * 128 : (kc + 1) * 128],
                                rhs=qT_sb[p0 : p0 + 64, t, qc * 512 : (qc + 1) * 512],
                                start=True,
                                stop=True,
                            )
                            nc.scalar.activation(
                                out=exps[half][:, kc, qc * 512 : (qc + 1) * 512],
                                in_=ps,
                                func=EXP,
                                scale=ATTN_SCALE,
                            )
                # AV + sumexp + normalize
                for half in range(2):
                    h = 2 * t + half
                    stage = small.tile([HD + 1, S], F32, tag="stage", name="stage")
                    for qc in range(2):
                        cps = cx.tile([HD + 1, 512], F32, tag="cx", name="cxps")
                        for kc in range(KC):
                            nc.tensor.matmul(
                                cps,
                                lhsT=v_sb[:, kc, h * VW : (h + 1) * VW],
                                rhs=exps[half][:, kc, qc * 512 : (qc + 1) * 512],
                                start=(kc == 0),
                                stop=(kc == KC - 1),
                            )
                        nc.vector.tensor_copy(
                            out=stage[:, qc * 512 : (qc + 1) * 512], in_=cps
                        )
                    # 1/sumexp: reshape the [1, S] row across 64 DVE lanes
                    r64 = small.tile([64, 16], F32, tag="r64", name="r64")
                    nc.sync.dma_start(out=r64, in_=stage[HD : HD + 1, :])
                    rc64 = small.tile([64, 16], F32, tag="rc64", name="rc64")
                    nc.vector.reciprocal(rc64, r64)
                    rrow = small.tile([1, S], F32, tag="rrow", name="rrow")
                    nc.sync.dma_start(out=rrow, in_=rc64)
                    bcast = small.tile([HD, S], F32, tag="bcast", name="bcast")
                    nc.gpsimd.partition_broadcast(bcast, rrow)
                    nc.vector.tensor_mul(
                        ctxT_sb[half * 64 : half * 64 + 64, t, :],
                        stage[0:HD, :],
                        bcast,
                    )

    xpool_cm.__exit__(None, None, None)

    # ---------------- phase O: output projection ----------------
    with ExitStack() as p3:
        wpool3 = p3.enter_context(tc.tile_pool(name="wpool3", bufs=3))
        opp = p3.enter_context(tc.tile_pool(name="opp", bufs=4, space="PSUM"))
        ostage = p3.enter_context(tc.tile_pool(name="ostage", bufs=4))
        for mg in range(MG):
            stripe = wpool3.tile([128, DT, 256], BF16, tag="w3", name=f"w3_{mg}")
            nc.sync.dma_start(
                out=stripe,
                in_=woT[:, mg * 256 : (mg + 1) * 256].rearrange(
                    "(t p) n -> p t n", p=128
                ),
            )
            for ml in range(2):
                m = mg * 2 + ml
                for qc in range(2):
                    ps = opp.tile([128, 512], F32, tag="opp", name="opps")
                    for kk in range(DT):
                        nc.tensor.matmul(
                            ps,
                            lhsT=stripe[:, kk, ml * 128 : (ml + 1) * 128],
                            rhs=ctxT_sb[:, kk, qc * 512 : (qc + 1) * 512],
                            start=(kk == 0),
                            stop=(kk == DT - 1),
                        )
                    o_sb = ostage.tile([128, 512], F32, tag="ostage", name="osb")
                    nc.vector.tensor_scalar_add(o_sb, ps, bo_sb[:, m, :])
                    nc.sync.dma_start(
                        out=outT[m * 128 : (m + 1) * 128, qc * 512 : (qc + 1) * 512],
                        in_=o_sb,
                    )
    persist_cm.__exit__(None, None, None)


def build_nc():
    nc = bacc.Bacc(None, target_bir_lowering=False)
    xT = nc.dram_tensor("xT", [D, S], BF16, kind="ExternalInput")
    wqT = nc.dram_tensor("wqT", [D, D], BF16, kind="ExternalInput")
    wkT = nc.dram_tensor("wkT", [D, D], BF16, kind="ExternalInput")
    wvT = nc.dram_tensor("wvT", [D, D], BF16, kind="ExternalInput")
    woT = nc.dram_tensor("woT", [D, D], BF16, kind="ExternalInput")
    bo = nc.dram_tensor("bo", [D], F32, kind="ExternalInput")
    outT = nc.dram_tensor("outT", [D, S], F32, kind="ExternalOutput")
    with tile.TileContext(nc) as tc:
        _emit(nc, tc, xT, wqT, wkT, wvT, woT, bo, outT)
    nc.compile()
    return nc


_NC = None


def _get_nc():
    global _NC
    if _NC is None:
        _NC = build_nc()
    return _NC


def make_in_maps(hidden_states, Wq, Wk, Wv, Wo, bo, Aq, Bq, Ak, Bk, Av, Bv, Ao, Bo):
    x = np.asarray(hidden_states, dtype=np.float32)

    def eff_T(W, A, Bup):
        W64 = np.asarray(W, dtype=np.float64)
        lora = np.asarray(Bup, dtype=np.float64) @ np.asarray(A, dtype=np.float64)
        return np.ascontiguousarray(
            (W64 + SCALING * lora).T.astype(ml_dtypes.bfloat16)
        )

    base = {
        "wqT": eff_T(Wq, Aq, Bq),
        "wkT": eff_T(Wk, Ak, Bk),
        "wvT": eff_T(Wv, Av, Bv),
        "woT": eff_T(Wo, Ao, Bo),
        "bo": np.ascontiguousarray(np.asarray(bo, dtype=np.float32)),
    }
    return [
        dict(base, xT=np.ascontiguousarray(x[b].T.astype(ml_dtypes.bfloat16)))
        for b in range(x.shape[0])
    ]


def kernel(**inputs):
    in_maps = make_in_maps(**inputs)
    nc = _get_nc()
    res = run_bass_kernel_spmd(nc, in_maps, core_ids=list(range(N_CORES)))
    out = np.stack([res.results[b]["outT"].T for b in range(N_CORES)])
    return np.ascontiguousarray(out, dtype=np.float32)
